# revision 20
# baseline (speedup 1.0000x reference)
"""BilinearAttention Trainium2 kernel.

Shapes (hardcoded): batch=8, la=lb=256, D=512, H=8, DH=64.
Sharding: data-parallel over batch -- core k computes batch element k
(all 8 heads), no collectives; gather on host.

Math per (b, h):
  A_ = relu(A @ W1.T + b1) row-major-reshaped to head mats Ah [256, 64]
  att[i, j]  = sum_c q_c * tanh(Ah[i,c] * Bh[j,c])
  temp_b2a   = softmax_i(mean_j att), temp_a2b = softmax_j(mean_i att)
  pooled     = sums of raw A/B head mats weighted by temps.

Device layout: channels-on-partitions (c stacked twice for a j-pair), so
the pairwise product is one DVE tensor_scalar per j-pair, tanh runs on
huge tiles on ScalarE, and the q-weighted channel reduction is a TensorE
matmul (tanh tile as stationary operand, q columns moving).  q is split
q_hi + q_lo (both bf16) to keep fp32-level accuracy with bf16 matmuls.
"""

import numpy as np
import ml_dtypes

BATCH = 8
L = 256          # la == lb
D = 512
H = 8
DH = 64          # D // H
NCORES = 8
TBLK = 32        # j-pairs per tanh buffer
NT = L // 2      # j-pairs per head (128)

_cache = {}
H_LOOP = H  # debug knob: heads processed in big loop


def _build():
    import concourse.bass as bass
    import concourse.tile as tile
    from concourse import mybir, bacc

    F32 = mybir.dt.float32
    BF16 = mybir.dt.bfloat16
    AF = mybir.ActivationFunctionType
    ALU = mybir.AluOpType
    AX = mybir.AxisListType

    nc = bacc.Bacc("TRN2", target_bir_lowering=False, debug=False,
                   num_devices=NCORES)

    # ---- DRAM I/O ----
    dAT = nc.dram_tensor("AT", [D, L], F32, kind="ExternalInput")
    dBT = nc.dram_tensor("BT", [D, L], F32, kind="ExternalInput")
    dAh = nc.dram_tensor("Ahead", [H, L, DH], F32, kind="ExternalInput")
    dBh = nc.dram_tensor("Bhead", [H, L, DH], F32, kind="ExternalInput")
    dW1T = nc.dram_tensor("W1T", [D, D], F32, kind="ExternalInput")
    dW2T = nc.dram_tensor("W2T", [D, D], F32, kind="ExternalInput")
    db1r = nc.dram_tensor("b1r", [128, 4], F32, kind="ExternalInput")
    db2r = nc.dram_tensor("b2r", [128, 4], F32, kind="ExternalInput")
    dQD = nc.dram_tensor("qdiag4", [128, 4], BF16, kind="ExternalInput")
    dOE = nc.dram_tensor("onesE", [128, 8 * H], BF16, kind="ExternalInput")
    dID = nc.dram_tensor("iden", [128, 128], F32, kind="ExternalInput")
    dMW = nc.dram_tensor("meanw", [H, 1], F32, kind="ExternalInput")
    dO1 = nc.dram_tensor("OUT1", [1, 2 * D], F32, kind="ExternalOutput")
    dO2 = nc.dram_tensor("OUT2", [1, L], F32, kind="ExternalOutput")
    dO3 = nc.dram_tensor("OUT3", [1, L], F32, kind="ExternalOutput")

    with tile.TileContext(nc) as tc:
        with tc.tile_pool(name="const", bufs=1) as cp, \
             tc.tile_pool(name="proj_ps", bufs=1, space="PSUM") as ppj, \
             tc.tile_pool(name="att_ps", bufs=4, space="PSUM") as pat, \
             tc.tile_pool(name="cs_ps", bufs=1, space="PSUM") as pcs, \
             tc.tile_pool(name="end_ps", bufs=1, space="PSUM") as pend, \
             tc.tile_pool(name="pb", bufs=3) as pb_pool, \
             tc.tile_pool(name="tb", bufs=3) as tb_pool, \
             tc.tile_pool(name="atts", bufs=3) as atts_pool, \
             tc.tile_pool(name="small", bufs=2) as sm:

            # ------- load constants / inputs -------
            w1t = []
            w2t = []
            at = []
            bt = []
            for kc in range(4):
                t = cp.tile([128, D], F32, tag=f"w1t{kc}")
                nc.sync.dma_start(t[:], dW1T.ap()[128 * kc:128 * (kc + 1), :])
                w1t.append(t)
                t = cp.tile([128, D], F32, tag=f"w2t{kc}")
                nc.sync.dma_start(t[:], dW2T.ap()[128 * kc:128 * (kc + 1), :])
                w2t.append(t)
                t = cp.tile([128, L], F32, tag=f"at{kc}")
                nc.sync.dma_start(t[:], dAT.ap()[128 * kc:128 * (kc + 1), :])
                at.append(t)
                t = cp.tile([128, L], F32, tag=f"bt{kc}")
                nc.sync.dma_start(t[:], dBT.ap()[128 * kc:128 * (kc + 1), :])
                bt.append(t)
            ah_raw = {}
            bh_raw = {}
            for h in range(H):
                for ih in range(2):
                    t = cp.tile([128, DH], F32, tag=f"ahr{h}_{ih}")
                    nc.sync.dma_start(t[:], dAh.ap()[h, 128 * ih:128 * (ih + 1), :])
                    ah_raw[(h, ih)] = t
                    t = cp.tile([128, DH], F32, tag=f"bhr{h}_{ih}")
                    nc.sync.dma_start(t[:], dBh.ap()[h, 128 * ih:128 * (ih + 1), :])
                    bh_raw[(h, ih)] = t
            b1r = cp.tile([128, 4], F32, tag="b1r")
            nc.sync.dma_start(b1r[:], db1r.ap())
            b2r = cp.tile([128, 4], F32, tag="b2r")
            nc.sync.dma_start(b2r[:], db2r.ap())
            qd = cp.tile([128, 4], BF16, tag="qd")
            nc.sync.dma_start(qd[:], dQD.ap())
            oe = cp.tile([128, 8 * H], BF16, tag="oe")
            nc.sync.dma_start(oe[:], dOE.ap())
            iden = cp.tile([128, 128], F32, tag="iden")
            nc.sync.dma_start(iden[:], dID.ap())
            mw = cp.tile([H, 1], F32, tag="mw")
            nc.sync.dma_start(mw[:], dMW.ap())

            # ------- projections: xhT2[c(+64), h*256 + i] , i = lloc*8 + dc -------
            ahT2 = cp.tile([128, H * L], BF16, tag="ahT2")
            bhT2 = cp.tile([128, H * L], BF16, tag="bhT2")
            for (wt, xt, br, dst) in ((w1t, at, b1r, ahT2),
                                      (w2t, bt, b2r, bhT2)):
                for half in range(2):
                    ps = ppj.tile([128, 2 * L], F32, tag="proj")
                    for mc2 in range(2):
                        mcc = 2 * half + mc2     # 128-row d_out chunk
                        for kc in range(4):
                            nc.tensor.matmul(
                                ps[:, L * mc2:L * (mc2 + 1)],
                                wt[kc][:, 128 * mcc:128 * (mcc + 1)],
                                xt[kc][:],
                                start=(kc == 0), stop=(kc == 3),
                            )
                    # relu + bias, scatter dc into i-stride-8 positions;
                    # upper 64 psum partitions shift down to channels 0:64
                    for mc2 in range(2):
                        mcc = 2 * half + mc2
                        for up in range(2):
                            mc = 2 * mcc + up    # dc index 0..7
                            out_v = dst[0:DH, :].rearrange(
                                "p (h l d) -> p h l d",
                                h=H, l=32, d=8)[:, :, :, mc]
                            in_v = ps[64 * up:64 * (up + 1),
                                      L * mc2:L * (mc2 + 1)].rearrange(
                                "p (h l) -> p h l", h=H)
                            nc.vector.tensor_scalar(
                                out_v, in_v,
                                br[64 * up:64 * (up + 1), mcc:mcc + 1], 0.0,
                                ALU.add, ALU.max)
                # duplicate channels to upper 64 partitions
                nc.vector.tensor_copy(dst[64:128, :], dst[0:DH, :])

            # ------- bcols[c-slot, h*128 + t] = BhT[c, 2t + slot] -------
            bcols = cp.tile([128, H * NT], F32, tag="bcols")
            bhT2_v = bhT2[:].rearrange("p (h j e) -> p h j e", h=H, e=2)
            for h in range(H):
                nc.vector.tensor_copy(bcols[0:64, NT * h:NT * (h + 1)],
                                      bhT2_v[0:64, h, :, 0])
                nc.vector.tensor_copy(bcols[64:128, NT * h:NT * (h + 1)],
                                      bhT2_v[64:128, h, :, 1])

            # ------- big loop -------
            rs_all = cp.tile([128, 2 * H], F32, tag="rs_all")  # row sums
            cs4 = pcs.tile([H, 4 * NT], F32, tag="cs4")        # col sums (hi/lo)
            first_cs = [True]
            for h in range(H_LOOP):
                attp0 = pat.tile([128, 4 * NT], F32, tag="attp")
                attp1 = pat.tile([128, 4 * NT], F32, tag="attp")
                attp = [attp0, attp1]
                for tbi in range(NT // TBLK):
                    pb = pb_pool.tile([128, TBLK * L], BF16, tag="pb")
                    for tt in range(TBLK):
                        t = TBLK * tbi + tt
                        nc.vector.tensor_scalar(
                            pb[:, L * tt:L * (tt + 1)],
                            ahT2[:, L * h:L * (h + 1)],
                            bcols[:, NT * h + t:NT * h + t + 1],
                            None, ALU.mult)
                    tbf = tb_pool.tile([128, TBLK * L], BF16, tag="tbf")
                    nc.scalar.activation(tbf[:], pb[:], AF.Tanh)
                    for tt in range(TBLK):
                        t = TBLK * tbi + tt
                        for ih in range(2):
                            nc.tensor.matmul(
                                attp[ih][:, 4 * t:4 * t + 4],
                                tbf[:, L * tt + 128 * ih:L * tt + 128 * (ih + 1)],
                                qd[:],
                                start=True, stop=True, skip_group_check=True)
                for ih in range(2):
                    atts = atts_pool.tile([128, 4 * NT], BF16, tag="atts")
                    nc.scalar.activation(
                        atts[:], attp[ih][:], AF.Copy,
                        accum_out=rs_all[:, 8 * ih + h:8 * ih + h + 1])
                    nc.tensor.matmul(
                        cs4[:], oe[:, 8 * h:8 * (h + 1)], atts[:],
                        start=first_cs[0], stop=(h == H_LOOP - 1 and ih == 1),
                        skip_group_check=True)
                    first_cs[0] = False

            # ------- softmaxes -------
            def softmax(src_ap, dst):
                nb = sm.tile([H, 1], F32, tag="nb")
                nc.vector.tensor_reduce(nb[:], src_ap, AX.X, ALU.max,
                                        negate=True)
                nbs = sm.tile([H, 1], F32, tag="nbs")
                nc.vector.tensor_scalar(nbs[:], nb[:], 1.0 / L, None, ALU.mult)
                e = sm.tile([H, L], F32, tag="sm_e")
                nc.scalar.activation(e[:], src_ap, AF.Exp, bias=nbs[:],
                                     scale=1.0 / L)
                s = sm.tile([H, 1], F32, tag="sm_s")
                nc.vector.tensor_reduce(s[:], e[:], AX.X, ALU.add)
                r = sm.tile([H, 1], F32, tag="sm_r")
                nc.vector.reciprocal(r[:], s[:])
                nc.vector.tensor_scalar(dst[:], e[:], r[:], None, ALU.mult)

            # b2a: transpose row sums [128, 16] -> [8, 256]
            ps_rs = pend.tile([H, L], F32, tag="end")
            nc.tensor.transpose(ps_rs[:, 0:128], rs_all[:, 0:H], iden[:])
            nc.tensor.transpose(ps_rs[:, 128:256], rs_all[:, H:2 * H], iden[:])
            t_b2a = sm.tile([H, L], F32, tag="t_b2a")
            softmax(ps_rs[:], t_b2a)

            # a2b: recombine hi+lo col sums
            cs4_sb = sm.tile([H, 4 * NT], F32, tag="cs4_sb")
            nc.vector.tensor_copy(cs4_sb[:], cs4[:])
            cs_sb = sm.tile([H, L], F32, tag="cs_sb")
            cs4_v = cs4_sb[:].rearrange("p (t e) -> p t e", e=4)
            nc.vector.tensor_add(cs_sb[:], cs4_v[:, :, 0:2], cs4_v[:, :, 2:4])
            t_a2b = sm.tile([H, L], F32, tag="t_a2b")
            softmax(cs_sb[:], t_a2b)

            # ------- out2 / out3: mean over heads -------
            for (tmp, dout, tag) in ((t_b2a, dO2, "o2"), (t_a2b, dO3, "o3")):
                ps_m = pend.tile([1, L], F32, tag="end")
                nc.tensor.matmul(ps_m[:], mw[:], tmp[:], start=True, stop=True,
                                 skip_group_check=True)
                o_sb = sm.tile([1, L], F32, tag=f"{tag}_sb")
                nc.vector.tensor_copy(o_sb[:], ps_m[:])
                nc.sync.dma_start(dout.ap(), o_sb[:])

            # ------- pooling -------
            # transpose temps -> weights on partitions: w_sb[i, 8*ih + h]
            for (tmp, raw, lo, tagp) in ((t_b2a, ah_raw, 0, "wA"),
                                         (t_a2b, bh_raw, D, "wB")):
                ps_w = pend.tile([128, 2 * H], F32, tag="end")
                nc.tensor.transpose(ps_w[:, 0:H], tmp[:, 0:128],
                                    iden[0:H, 0:H])
                nc.tensor.transpose(ps_w[:, H:2 * H], tmp[:, 128:256],
                                    iden[0:H, 0:H])
                w_sb = sm.tile([128, 2 * H], F32, tag=f"wsb_{tagp}")
                nc.vector.tensor_copy(w_sb[:], ps_w[:])
                ps_p = pend.tile([1, D], F32, tag="end")
                for h in range(H):
                    for ih in range(2):
                        nc.tensor.matmul(
                            ps_p[:, DH * h:DH * (h + 1)],
                            w_sb[:, 8 * ih + h:8 * ih + h + 1],
                            raw[(h, ih)][:],
                            start=(ih == 0), stop=(ih == 1),
                            skip_group_check=True)
                p_sb = sm.tile([1, D], F32, tag=f"psb_{tagp}")
                nc.vector.tensor_copy(p_sb[:], ps_p[:])
                nc.sync.dma_start(dO1.ap()[:, lo:lo + D], p_sb[:])

    nc.compile()
    return nc


def _prep_maps(A, B, W1, b1, W2, b2, q):
    bf = ml_dtypes.bfloat16
    W1T = np.ascontiguousarray(W1.T, dtype=np.float32)
    W2T = np.ascontiguousarray(W2.T, dtype=np.float32)
    b1r = np.ascontiguousarray(np.asarray(b1).reshape(4, 128).T,
                               dtype=np.float32)
    b2r = np.ascontiguousarray(np.asarray(b2).reshape(4, 128).T,
                               dtype=np.float32)
    q = q.astype(np.float32)
    q_hi = q.astype(bf).astype(np.float32)
    q_lo = (q - q_hi).astype(np.float32)
    qd = np.zeros((128, 4), dtype=np.float32)
    qd[0:64, 0] = q_hi
    qd[64:128, 1] = q_hi
    qd[0:64, 2] = q_lo
    qd[64:128, 3] = q_lo
    qd = qd.astype(bf)
    oe = np.zeros((128, 8 * H), dtype=np.float32)
    for h in range(H):
        oe[:, 8 * h + h] = 1.0
    oe = oe.astype(bf)
    iden = np.eye(128, dtype=np.float32)
    mw = np.full((H, 1), 1.0 / H, dtype=np.float32)

    maps = []
    for k in range(NCORES):
        Ak = np.ascontiguousarray(A[k], dtype=np.float32)
        Bk = np.ascontiguousarray(B[k], dtype=np.float32)
        maps.append({
            "AT": np.ascontiguousarray(Ak.T),
            "BT": np.ascontiguousarray(Bk.T),
            "Ahead": np.ascontiguousarray(Ak.reshape(H, L, DH)),
            "Bhead": np.ascontiguousarray(Bk.reshape(H, L, DH)),
            "W1T": W1T, "W2T": W2T, "b1r": b1r, "b2r": b2r,
            "qdiag4": qd, "onesE": oe, "iden": iden, "meanw": mw,
        })
    return maps


def _install_ntff_shim():
    """Register the antenv.axon_hooks NTFF profiling shim (dev-only).

    The agent image's antenv lacks axon_hooks; rebuild it from the boot
    helper so run_bass_kernel_spmd(trace=True) can capture NTFF profiles.
    """
    import sys
    import types
    if "antenv.axon_hooks" in sys.modules:
        return
    try:
        sys.path.insert(0, "/root/.axon_site/trn_agent_boot")
        import trn_fixups  # noqa: F401  (package presence check)
        import importlib
        trn_boot = importlib.import_module("trn_boot")
        hook = trn_boot._ntff_profile_via_ctypes("/opt/axon/libaxon_pjrt.so")
        mod = types.ModuleType("antenv.axon_hooks")
        mod._hook = hook
        mod.get_axon_ntff_profile_hook = lambda: mod._hook
        mod.set_axon_ntff_profile_hook = lambda h: setattr(mod, "_hook", h)
        sys.modules["antenv.axon_hooks"] = mod
    except Exception as e:  # profiling is best-effort
        print("ntff shim failed:", e)


def kernel(A, B, W1, b1, W2, b2, q, _trace=False):
    from concourse.bass_utils import run_bass_kernel_spmd

    if _trace:
        _install_ntff_shim()

    if "nc" not in _cache:
        _cache["nc"] = _build()
    nc = _cache["nc"]
    maps = _prep_maps(A, B, W1, b1, W2, b2, q)
    res = run_bass_kernel_spmd(nc, maps, core_ids=list(range(NCORES)),
                               trace=_trace)
    _cache["last_results"] = res
    out1 = np.concatenate([res.results[k]["OUT1"] for k in range(NCORES)])
    out2 = np.concatenate([res.results[k]["OUT2"] for k in range(NCORES)])
    out3 = np.concatenate([res.results[k]["OUT3"] for k in range(NCORES)])
    return out1, out2, out3
